# revision 2
# baseline (speedup 1.0000x reference)
"""RGCN (3 RelGraphConv layers + mean-pool + MLP + softmax) on 8 Trainium2 cores.

v2 strategy (dst-node sharding + fp8 DoubleRow matmuls):
  - Core c owns dst nodes [c*6250, (c+1)*6250), 25 tiles of 256.
  - h tables live in DRAM as fp8e4 rows padded to 256 B (dma_gather granularity).
  - Per tile, edges are grouped by (half, relation) into 128-slot columns with a
    SYMMETRIC layout: each relation gets the same number of columns in the A
    (src < thr) and B (src >= thr) blocks, so column j of A pairs with column j
    of B at a constant stride. One fp16 one-hot (DVE is_equal, fast 2x mode)
    serves both planes of a fp8 DoubleRow scatter matmul: the fp16 buffer is
    bitcast to fp8 byte pairs whose high bytes are 0x3C (= 1.5 in e4m3) at the
    one-hot positions. The 1.5 factor is divided out in the PSUM->SBUF copy.
  - The per-relation aggregates (fp8) hit the relation weights with DoubleRow
    matmuls too: relations are paired (0,1)(2,3)..(14,15)(loop,0) as the two
    contraction planes; W is host-packed fp8e4 (x32 prescale, undone in the
    ReLU activation scale) in plane-interleaved pairs.
  - dma_gather runs one call per (tile, half) chunk (single_packet=False,
    48 KB SWDGE ring) to amortize the 994 ns fixed SWDGE cost.
  - Layer-2 output stays fp16 and feeds weighted one-hot pooling matmuls
    accumulated in PSUM; AllReduce combines cores; transposed fp32 MLP +
    PE transpose + free-dim softmax emit [128, 8] on every core.
  - AllGather replicates fp8 node tables between conv layers.
"""

import sys

sys.path.insert(0, "/opt/trn_rl_repo")

import math
import numpy as np
import ml_dtypes

from concourse import bass, bacc, mybir, tile
from concourse import bass_utils

F32 = mybir.dt.float32
F16 = mybir.dt.float16
F8 = mybir.dt.float8e4
I16 = mybir.dt.int16
NPF8 = ml_dtypes.float8_e4m3

TS = 256          # dst nodes per tile
ROWB = 256        # fp8 elements per padded table row (256 B)
CHUNK_COLS = 22   # max columns per dma_gather call (ring = 3072 descs)
W_SCALE = 32.0    # host premultiplier on conv weights (undone in ReLU scale)
H_SCALE = 1.5     # e4m3 value of the one-hot's high byte (undone in at-copy)


class Cfg:
    def __init__(self, N, R, G, NC, cores, conv, mlp, split_cap=32768):
        self.N = N
        self.R = R
        self.G = G
        self.NC = NC
        self.cores = cores
        self.conv = conv
        self.mlp = mlp
        self.NPC = N // cores
        self.T = math.ceil(self.NPC / TS)
        self.split_cap = min(split_cap, N)
        self.baseB = max(0, N - self.split_cap)
        self.split_thr = min(max(N // 2, self.baseB), self.split_cap)


FULL_CFG = Cfg(
    N=50000, R=16, G=128, NC=8, cores=8,
    conv=[(128, 128), (128, 256), (256, 128)],
    mlp=[(128, 128), (128, 256), (256, 128)],
)


def _preprocess_edges(src, dst, rel, cfg):
    """Symmetric column layout + per-core slot arrays.

    Per tile: [A-cols r0..r16][B-cols r0..r16] with nc_sym[t,r] columns per
    relation IN EACH half (>=1), so A col j pairs with B col j at stride nA_t.
    """
    C, NPC, T, R = cfg.cores, cfg.NPC, cfg.T, cfg.R
    NG = R + 1
    rows_t = np.minimum(NPC - np.arange(T) * TS, TS).astype(np.int64)

    owner = dst // NPC
    isB = (src >= cfg.split_thr).astype(np.int64)
    tloc = (dst % NPC) // TS
    key_full = ((owner * T + tloc) * 2 + isB) * NG + rel
    cnt = np.bincount(key_full, minlength=C * T * 2 * NG).reshape(C, T, 2, NG)
    for t in range(T):
        for c in range(C):
            vs = c * NPC + t * TS + np.arange(int(rows_t[t]))
            nb = int((vs >= cfg.split_thr).sum())
            cnt[c, t, 0, R] = len(vs) - nb
            cnt[c, t, 1, R] = nb

    # symmetric per-half column count, min 1
    nc_sym = np.maximum(1, -(-cnt.max(axis=(0, 2)) // 128))  # [T, NG]
    a0 = np.zeros((T, NG), np.int64)  # local col offset of rel r's A block
    for t in range(T):
        a0[t] = np.concatenate([[0], np.cumsum(nc_sym[t])[:-1]])
    nA_t = nc_sym.sum(axis=1)  # [T]
    Ct_t = 2 * nA_t
    tile_base = np.zeros(T + 1, np.int64)
    tile_base[1:] = np.cumsum(Ct_t)
    NCOL = int(tile_base[-1])

    idx_po = np.zeros((C, 128, NCOL), np.int16)
    dstl_po = np.full((C, 128, NCOL), -1.0, np.float16)
    for c in range(C):
        m = owner == c
        es, ed, er = src[m], dst[m], rel[m]
        eb = (es >= cfg.split_thr).astype(np.int64)
        tl = (ed - c * NPC) // TS
        dl = (ed - c * NPC) % TS
        k = (tl * 2 + eb) * NG + er
        order = np.argsort(k, kind="stable")
        es, dl, k, eb = es[order], dl[order], k[order], eb[order]
        grp_start = np.searchsorted(k, np.arange(T * 2 * NG))
        j = np.arange(len(k)) - grp_start[k]
        kt, krem = k // (2 * NG), k % (2 * NG)
        kh, kr = krem // NG, krem % NG
        col = tile_base[kt] + kh * nA_t[kt] + a0[kt, kr] + j // 128
        p = j % 128
        val = np.where(eb == 1, es - cfg.baseB, es).astype(np.int16)
        idx_po[c, p, col] = val
        dstl_po[c, p, col] = dl.astype(np.float16)
        # self edges (r = R)
        for t in range(T):
            vl = np.arange(int(rows_t[t]))
            vg = c * NPC + t * TS + vl
            for h in range(2):
                sel = (vg >= cfg.split_thr) == (h == 1)
                if not sel.any():
                    continue
                vv, ll = vg[sel], vl[sel]
                cs = tile_base[t] + h * nA_t[t] + a0[t, R]
                jj = np.arange(len(vv))
                vval = vv - cfg.baseB if h == 1 else vv
                idx_po[c, jj % 128, cs + jj // 128] = vval.astype(np.int16)
                dstl_po[c, jj % 128, cs + jj // 128] = ll.astype(np.float16)

    # 16-wrapped idx array; per (t, half, chunk) call covers contiguous cols
    idx_w = np.zeros((C, 128, NCOL * 8), np.int16)
    calls = []  # (t, half, local_col0, ncols)
    for t in range(T):
        for h in range(2):
            blk0 = int(tile_base[t] + h * nA_t[t])
            nb = int(nA_t[t])
            for q0 in range(0, nb, CHUNK_COLS):
                calls.append((t, h, q0, min(CHUNK_COLS, nb - q0)))
            for c in range(C):
                flat = idx_po[c][:, blk0:blk0 + nb].T.ravel()
                blk = flat.reshape(-1, 16).T
                idx_w[c, :, blk0 * 8:blk0 * 8 + len(flat) // 16] = np.tile(
                    blk, (8, 1))

    layout = {
        "NCOL": NCOL,
        "nc_sym": nc_sym,        # [T, NG]
        "a0": a0,                # [T, NG]
        "nA_t": nA_t,
        "tile_base": tile_base,
        "rows_t": rows_t,
        "Cmax": int(Ct_t.max()),
        "calls": calls,
    }
    return layout, idx_w, dstl_po


def _pack_weights(inputs, cfg):
    """Conv weights: fp8e4, x32, relation-pair plane-interleaved.

    Wcv{l}: [ks, 128, 9*2*do] where pair q holds (W_{2q}, W_{2q+1}) for q<8
    and (W_loop, 0) for q=8, laid out [q][plane][do].
    """
    R = cfg.R
    packed = {}
    for l, (di, do) in enumerate(cfg.conv):
        ks = di // 128
        npair = (R + 2) // 2  # 9
        Wp = np.zeros((ks, 128, npair * 2 * do), NPF8)
        Wr = np.asarray(inputs[f"W_rel{l}"], np.float32) * W_SCALE
        Wl = np.asarray(inputs[f"W_loop{l}"], np.float32) * W_SCALE
        for k in range(ks):
            for q in range(npair):
                for tt in range(2):
                    r = 2 * q + tt
                    if r < R:
                        w = Wr[r, k * 128:(k + 1) * 128, :]
                    elif r == R:
                        w = Wl[k * 128:(k + 1) * 128, :]
                    else:
                        continue  # zero plane
                    Wp[k, :, (q * 2 + tt) * do:(q * 2 + tt + 1) * do] = (
                        w.astype(NPF8))
        packed[f"Wcv{l}"] = Wp
        packed[f"bcv{l}"] = (np.asarray(inputs[f"b{l}"], np.float32)
                             * W_SCALE).astype(np.float16).reshape(1, do)
    for l, (di, do) in enumerate(cfg.mlp):
        ks = di // 128
        W = np.asarray(inputs[f"Wh{l}"], np.float32)
        packed[f"Wm{l}"] = W.reshape(ks, 128, do)
        nmt = -(-do // 128)
        bp = np.zeros((nmt, 128, 1), np.float32)
        b = np.asarray(inputs[f"bh{l}"], np.float32)
        for mi in range(nmt):
            seg = b[mi * 128:(mi + 1) * 128]
            bp[mi, :len(seg), 0] = seg
        packed[f"bm{l}"] = bp
    packed["Wcls"] = np.asarray(inputs["Wc"], np.float32).reshape(1, 128, cfg.NC)
    packed["bcls"] = np.asarray(inputs["bc"], np.float32).reshape(cfg.NC, 1)
    return packed


def _pool_arrays(graph_ids, cfg):
    C, NPC = cfg.cores, cfg.NPC
    ST = math.ceil(NPC / 128)
    cnts = np.bincount(graph_ids, minlength=cfg.G).astype(np.float64)
    wg = (1.0 / np.maximum(cnts, 1.0)).astype(np.float32)
    gid_po = np.full((C, 128, ST), -1.0, np.float16)
    wnd_po = np.zeros((C, 128, ST), np.float32)
    for c in range(C):
        for st in range(ST):
            nt = min(128, NPC - st * 128)
            if nt <= 0:
                continue
            v = c * NPC + st * 128 + np.arange(nt)
            gid_po[c, :nt, st] = graph_ids[v].astype(np.float16)
            wnd_po[c, :nt, st] = wg[graph_ids[v]]
    return gid_po, wnd_po


def build_program(cfg, layout, debug=False, timing=False):
    nc = bacc.Bacc(
        "TRN2", target_bir_lowering=False, debug=False,
        enable_asserts=False, num_devices=cfg.cores,
        dynamic_dma_scratch_size=49152,
    )
    R, T, G, NC = cfg.R, cfg.T, cfg.G, cfg.NC
    NG = R + 1
    NPAIR = (R + 2) // 2
    NCOL, Cmax = layout["NCOL"], layout["Cmax"]
    nc_sym, a0 = layout["nc_sym"], layout["a0"]
    tile_base, nA_t = layout["tile_base"], layout["nA_t"]
    rows_t, calls = layout["rows_t"], layout["calls"]
    ST = math.ceil(cfg.NPC / 128)
    DR = mybir.MatmulPerfMode.DoubleRow

    h0 = nc.dram_tensor("h0", [cfg.N, ROWB], F8, kind="ExternalInput")
    idxT = nc.dram_tensor("idxw", [128, NCOL * 8], I16, kind="ExternalInput")
    dstlT = nc.dram_tensor("dstl", [128, NCOL], F16, kind="ExternalInput")
    gidT = nc.dram_tensor("gid", [128, ST], F16, kind="ExternalInput")
    wndT = nc.dram_tensor("wnd", [128, ST], F32, kind="ExternalInput")
    iotaT = nc.dram_tensor("iota", [128, Cmax * TS], F16, kind="ExternalInput")
    onesT = nc.dram_tensor("ones1", [1, 128], F16, kind="ExternalInput")
    idcT = nc.dram_tensor("idc", [NC, NC], F32, kind="ExternalInput")
    WcvT, bcvT = [], []
    for l, (di, do) in enumerate(cfg.conv):
        WcvT.append(nc.dram_tensor(f"Wcv{l}", [di // 128, 128, NPAIR * 2 * do],
                                   F8, kind="ExternalInput"))
        bcvT.append(nc.dram_tensor(f"bcv{l}", [1, do], F16, kind="ExternalInput"))
    WmT, bmT = [], []
    for l, (di, do) in enumerate(cfg.mlp):
        WmT.append(nc.dram_tensor(f"Wm{l}", [di // 128, 128, do], F32,
                                  kind="ExternalInput"))
        bmT.append(nc.dram_tensor(f"bm{l}", [-(-do // 128), 128, 1], F32,
                                  kind="ExternalInput"))
    WclsT = nc.dram_tensor("Wcls", [1, 128, NC], F32, kind="ExternalInput")
    bclsT = nc.dram_tensor("bcls", [NC, 1], F32, kind="ExternalInput")
    outT = nc.dram_tensor("out", [G, NC], F32, kind="ExternalOutput")

    h_full = [h0]
    ag_in = []
    for l in range(2):
        ag_in.append(nc.dram_tensor(f"agin{l}", [cfg.NPC, ROWB], F8))
        h_full.append(nc.dram_tensor(f"hfull{l + 1}", [cfg.N, ROWB], F8))
    pool_in = nc.dram_tensor("plin", [128, G], F32)
    pool_out = nc.dram_tensor("plout", [128, G], F32)
    dbg = {}
    if debug:
        dbg["h1"] = nc.dram_tensor("dbg_h1", [cfg.N, ROWB], F8,
                                   kind="ExternalOutput")
        dbg["h2"] = nc.dram_tensor("dbg_h2", [cfg.N, ROWB], F8,
                                   kind="ExternalOutput")
        dbg["pool"] = nc.dram_tensor("dbg_pool", [128, G], F32,
                                     kind="ExternalOutput")

    rg = [list(range(cfg.cores))]
    calls_by_tile = {}
    for (t, h, q0, ncq) in calls:
        calls_by_tile.setdefault(t, []).append((h, q0, ncq))

    with tile.TileContext(nc) as tc:
        with (
            tc.tile_pool(name="const", bufs=1) as cp,
            tc.tile_pool(name="wp", bufs=1) as wp,
            tc.tile_pool(name="gp", bufs=3) as gp,
            tc.tile_pool(name="hp", bufs=2) as hp,
            tc.tile_pool(name="atp", bufs=3) as atp,
            tc.tile_pool(name="hnp", bufs=4) as hnp,
            tc.tile_pool(name="mp", bufs=2) as mp,
            tc.tile_pool(name="psA", bufs=2, space="PSUM") as psA,
            tc.tile_pool(name="psG", bufs=2, space="PSUM") as psG,
            tc.tile_pool(name="psP", bufs=1, space="PSUM") as psP,
        ):
            idx_sb = cp.tile([128, NCOL * 8], I16)
            nc.sync.dma_start(out=idx_sb[:], in_=idxT[:, :])
            dstl_sb = cp.tile([128, NCOL], F16)
            nc.sync.dma_start(out=dstl_sb[:], in_=dstlT[:, :])
            iota_sb = cp.tile([128, Cmax * TS], F16)
            nc.sync.dma_start(out=iota_sb[:], in_=iotaT[:, :])
            gid_sb = cp.tile([128, ST], F16)
            nc.sync.dma_start(out=gid_sb[:], in_=gidT[:, :])
            wnd_sb = cp.tile([128, ST], F32)
            nc.sync.dma_start(out=wnd_sb[:], in_=wndT[:, :])
            ones_sb = cp.tile([1, 128], F16)
            nc.sync.dma_start(out=ones_sb[:], in_=onesT[:, :])
            idc_sb = cp.tile([NC, NC], F32)
            nc.sync.dma_start(out=idc_sb[:], in_=idcT[:, :])

            Wsb, bsb = [], []
            for l, (di, do) in enumerate(cfg.conv):
                ks = di // 128
                Wk = []
                for k in range(ks):
                    w = wp.tile([128, NPAIR * 2 * do], F8, tag=f"wcv{l}_{k}")
                    nc.sync.dma_start(out=w[:], in_=WcvT[l][k, :, :])
                    Wk.append(w)
                Wsb.append(Wk)
                b = wp.tile([1, do], F16, tag=f"bcv{l}")
                nc.sync.dma_start(out=b[:], in_=bcvT[l][:, :])
                bsb.append(b)
            Wm_sb, bm_sb = [], []
            for l, (di, do) in enumerate(cfg.mlp):
                ks = di // 128
                Wk = []
                for k in range(ks):
                    w = wp.tile([128, do], F32, tag=f"wm{l}_{k}")
                    nc.sync.dma_start(out=w[:], in_=WmT[l][k, :, :])
                    Wk.append(w)
                Wm_sb.append(Wk)
                nmt = -(-do // 128)
                bk = []
                for mi in range(nmt):
                    b = wp.tile([128, 1], F32, tag=f"bm{l}_{mi}")
                    nc.sync.dma_start(out=b[:], in_=bmT[l][mi, :, :])
                    bk.append(b)
                bm_sb.append(bk)
            Wcls_sb = wp.tile([128, NC], F32, tag="wcls")
            nc.sync.dma_start(out=Wcls_sb[:], in_=WclsT[0, :, :])
            bcls_sb = wp.tile([NC, 1], F32, tag="bcls")
            nc.sync.dma_start(out=bcls_sb[:], in_=bclsT[:, :])

            pool_ps = None

            for l, (di, do) in enumerate(cfg.conv):
                ks = di // 128
                gpb = 4 // ks  # groups per PSUM batch (pa = [128, 1024] f32)
                src_dram = h_full[l]
                tblA = src_dram[0:cfg.split_cap, :]
                tblB = src_dram[cfg.baseB:cfg.N, :]
                if l == 2:
                    pool_ps = psP.tile([128, G], F32, tag="pool")
                for t in range(T):
                    cb = int(tile_base[t])
                    nA = int(nA_t[t])
                    Ct = 2 * nA
                    rows = int(rows_t[t])
                    rows_ns = [min(128, rows), max(0, rows - 128)]

                    g_sb = gp.tile([128, Cmax * ROWB], F8, tag="g")
                    for (h, q0, ncq) in calls_by_tile[t]:
                        c_loc = h * nA + q0
                        n_idx = ncq * 128
                        o0 = (cb + c_loc) * 8
                        nc.gpsimd.dma_gather(
                            out_ap=g_sb[:, c_loc * ROWB:(c_loc + ncq) * ROWB]
                            .rearrange("p (c j) -> p c j", j=ROWB),
                            in_ap=(tblA if h == 0 else tblB),
                            idxs_ap=idx_sb[:, o0:o0 + n_idx // 16],
                            num_idxs=n_idx,
                            num_idxs_reg=n_idx,
                            elem_size=ROWB,
                            single_packet=False,
                        )
                    h_all = hp.tile([128, Cmax * TS], F16, tag="h")
                    nc.vector.tensor_tensor(
                        out=h_all[:, :Ct * TS].rearrange(
                            "p (c v) -> p c v", v=TS),
                        in0=iota_sb[:, :Ct * TS].rearrange(
                            "p (c v) -> p c v", v=TS),
                        in1=dstl_sb[:, cb:cb + Ct][:, :, None]
                        .to_broadcast([128, Ct, TS]),
                        op=mybir.AluOpType.is_equal,
                    )
                    # paired views: dim1 = half (A/B plane)
                    g2 = g_sb[:, :Ct * ROWB].rearrange(
                        "p (h x) -> p h x", h=2)
                    h8 = h_all[:, :Ct * TS].bitcast(F8).rearrange(
                        "p (h c v b) -> p h c v b", h=2, c=nA, v=TS, b=2)

                    agg = psG.tile([128, 512], F32, tag="agg")
                    for ns in range(2):
                        if rows_ns[ns] > 0:
                            nc.tensor.matmul(
                                out=agg[:, ns * do:ns * do + do],
                                lhsT=ones_sb[:1, :], rhs=bsb[l][:1, :],
                                start=True, stop=False,
                            )
                    bases = list(range(0, NG, gpb))
                    for bi, base in enumerate(bases):
                        bn = min(gpb, NG - base)
                        pa = psA.tile([128, 1024], F32, tag="pa")
                        for gi in range(bn):
                            r = base + gi
                            ncr = int(nc_sym[t, r])
                            ca0 = int(a0[t, r])
                            for k in range(ks):
                                for j in range(ncr):
                                    ca = ca0 + j
                                    nc.tensor.matmul(
                                        out=pa[:, gi * ks * TS + k * TS:
                                               gi * ks * TS + (k + 1) * TS],
                                        lhsT=g2[:, :, ca * ROWB + k * 128:
                                                ca * ROWB + k * 128 + 128],
                                        rhs=h8[:, :, ca, :, 1],
                                        start=(j == 0), stop=(j == ncr - 1),
                                        perf_mode=DR,
                                    )
                        wid = bn * ks * TS
                        at = atp.tile([128, 1024], F8, tag="at")
                        nc.scalar.activation(
                            out=at[:, :wid], in_=pa[:, :wid],
                            func=mybir.ActivationFunctionType.Copy,
                            scale=float(1.0 / H_SCALE),
                        )
                        at_v = at[:].rearrange("p (g x) -> p g x", x=ks * TS)
                        npair_b = (bn + 1) // 2
                        for q in range(npair_b):
                            qglob = (base + 2 * q) // 2
                            paired = 2 * q + 1 < bn
                            for ns in range(2):
                                if rows_ns[ns] == 0:
                                    continue
                                for k in range(ks):
                                    sl = k * TS + ns * 128
                                    if paired:
                                        lhsT = at_v[:, 2 * q:2 * q + 2,
                                                    sl:sl + 128]
                                    else:
                                        lhsT = at_v[:, 2 * q, sl:sl + 128][
                                            :, None, :].to_broadcast(
                                            [128, 2, 128])
                                    rhs = Wsb[l][k][:].rearrange(
                                        "p (q two d) -> p q two d",
                                        q=NPAIR, two=2)[:, qglob, :, :]
                                    last = (bi == len(bases) - 1
                                            and q == npair_b - 1
                                            and k == ks - 1)
                                    nc.tensor.matmul(
                                        out=agg[:, ns * do:ns * do + do],
                                        lhsT=lhsT, rhs=rhs,
                                        start=False, stop=last,
                                        perf_mode=DR,
                                    )
                    for ns in range(2):
                        rns = rows_ns[ns]
                        if rns == 0:
                            continue
                        st = t * 2 + ns
                        if l < 2:
                            hn = hnp.tile([128, 256], F8, tag="hn")
                            nc.scalar.activation(
                                out=hn[:, :do], in_=agg[:, ns * do:ns * do + do],
                                func=mybir.ActivationFunctionType.Relu,
                                scale=float(1.0 / W_SCALE),
                            )
                            nc.sync.dma_start(
                                out=ag_in[l][st * 128:st * 128 + rns, 0:do],
                                in_=hn[:rns, :do],
                            )
                        else:
                            hn16 = hnp.tile([128, 128], F16, tag="hn16")
                            nc.scalar.activation(
                                out=hn16[:, :do],
                                in_=agg[:, ns * do:ns * do + do],
                                func=mybir.ActivationFunctionType.Relu,
                                scale=float(1.0 / W_SCALE),
                            )
                            hg = mp.tile([128, G], F16, tag="hg")
                            nc.vector.tensor_tensor(
                                out=hg[:],
                                in0=iota_sb[:, :G],
                                in1=gid_sb[:, st:st + 1].to_broadcast([128, G]),
                                op=mybir.AluOpType.is_equal,
                            )
                            nc.vector.tensor_scalar_mul(
                                out=hg[:], in0=hg[:], scalar1=wnd_sb[:, st:st + 1]
                            )
                            nc.tensor.matmul(
                                out=pool_ps[:], lhsT=hn16[:, :do], rhs=hg[:],
                                start=(st == 0), stop=(st == 2 * T - 2),
                            )
                if l < 2:
                    if timing:
                        nc.sync.dma_start(
                            out=h_full[l + 1][0:cfg.NPC, :], in_=ag_in[l][:, :]
                        )
                    else:
                        nc.gpsimd.collective_compute(
                            "AllGather",
                            mybir.AluOpType.bypass,
                            replica_groups=rg,
                            ins=[ag_in[l].ap().opt()],
                            outs=[h_full[l + 1].ap().opt()],
                        )
                    if debug:
                        nc.sync.dma_start(
                            out=dbg[f"h{l + 1}"][:, :], in_=h_full[l + 1][:, :]
                        )

            # ---- pooled AllReduce + MLP (transposed, fp32) ----
            pl_sb = mp.tile([128, G], F32, tag="pl")
            nc.vector.tensor_copy(out=pl_sb[:], in_=pool_ps[:])
            nc.sync.dma_start(out=pool_in[:, :], in_=pl_sb[:])
            if timing:
                nc.sync.dma_start(out=pool_out[:, :], in_=pool_in[:, :])
            else:
                nc.gpsimd.collective_compute(
                    "AllReduce",
                    mybir.AluOpType.add,
                    replica_groups=rg,
                    ins=[pool_in.ap().opt()],
                    outs=[pool_out.ap().opt()],
                )
            hgT = mp.tile([128, G], F32, tag="hgt")
            nc.sync.dma_start(out=hgT[:], in_=pool_out[:, :])
            if debug:
                nc.sync.dma_start(out=dbg["pool"][:, :], in_=pool_out[:, :])

            cur = [hgT]
            for l, (di, do) in enumerate(cfg.mlp):
                ks = di // 128
                nmt = -(-do // 128)
                nxt = []
                for mi in range(nmt):
                    mw = min(128, do - mi * 128)
                    ps = psG.tile([128, 512], F32, tag="agg")
                    for k in range(ks):
                        nc.tensor.matmul(
                            out=ps[:mw, :G],
                            lhsT=Wm_sb[l][k][:, mi * 128:mi * 128 + mw],
                            rhs=cur[k][:],
                            start=(k == 0), stop=(k == ks - 1),
                        )
                    hx = mp.tile([128, G], F32, tag=f"mlph{l}_{mi}")
                    nc.scalar.activation(
                        out=hx[:mw, :], in_=ps[:mw, :G],
                        func=mybir.ActivationFunctionType.Relu,
                        bias=bm_sb[l][mi][:mw, :1],
                    )
                    nxt.append(hx)
                cur = nxt

            ps_cls = psG.tile([128, 512], F32, tag="agg")
            nc.tensor.matmul(
                out=ps_cls[:NC, :G], lhsT=Wcls_sb[:, :NC], rhs=cur[0][:],
                start=True, stop=True,
            )
            lgT = mp.tile([NC, G], F32, tag="lgT")
            nc.vector.tensor_scalar_add(
                out=lgT[:], in0=ps_cls[:NC, :G], scalar1=bcls_sb[:, :1]
            )
            ps_tr = psG.tile([128, 512], F32, tag="agg")
            nc.tensor.transpose(out=ps_tr[:G, :NC], in_=lgT[:], identity=idc_sb[:])
            lg = mp.tile([G, NC], F32, tag="lg")
            nc.vector.tensor_copy(out=lg[:], in_=ps_tr[:G, :NC])
            mx = mp.tile([G, 1], F32, tag="mx")
            nc.vector.tensor_reduce(
                out=mx[:], in_=lg[:], axis=mybir.AxisListType.X,
                op=mybir.AluOpType.max,
            )
            nc.vector.tensor_scalar_mul(out=mx[:], in0=mx[:], scalar1=-1.0)
            ex = mp.tile([G, NC], F32, tag="ex")
            nc.scalar.activation(
                out=ex[:], in_=lg[:], func=mybir.ActivationFunctionType.Exp,
                bias=mx[:, :1],
            )
            sm = mp.tile([G, 1], F32, tag="sm")
            nc.vector.tensor_reduce(
                out=sm[:], in_=ex[:], axis=mybir.AxisListType.X,
                op=mybir.AluOpType.add,
            )
            rs = mp.tile([G, 1], F32, tag="rs")
            nc.vector.reciprocal(out=rs[:], in_=sm[:])
            ot = mp.tile([G, NC], F32, tag="ot")
            nc.vector.tensor_scalar_mul(out=ot[:], in0=ex[:], scalar1=rs[:, :1])
            nc.sync.dma_start(out=outT[:, :], in_=ot[:])

    nc.compile()
    return nc


def make_in_maps(inputs, cfg, layout, idx_w, dstl_po):
    gid_po, wnd_po = _pool_arrays(
        np.asarray(inputs["graph_ids"]).astype(np.int64), cfg
    )
    packed = _pack_weights(inputs, cfg)
    Cmax = max(layout["Cmax"], 1)
    iota = np.tile(np.arange(TS, dtype=np.float16)[None, :], (128, Cmax))
    iota = iota.reshape(128, Cmax * TS)
    h8 = np.zeros((cfg.N, ROWB), NPF8)
    h8[:, :cfg.conv[0][0]] = np.asarray(inputs["h"], np.float32).astype(NPF8)
    shared = {
        "h0": h8,
        "iota": iota,
        "ones1": np.ones((1, 128), np.float16),
        "idc": np.eye(cfg.NC, dtype=np.float32),
    }
    shared.update(packed)
    in_maps = []
    for c in range(cfg.cores):
        m = dict(shared)
        m["idxw"] = idx_w[c]
        m["dstl"] = dstl_po[c]
        m["gid"] = gid_po[c]
        m["wnd"] = wnd_po[c]
        in_maps.append(m)
    return in_maps


_CACHE = {}
last_results = None


def _run(inputs, cfg, trace=False, debug=False):
    global last_results
    src = np.asarray(inputs["src"]).astype(np.int64)
    dst = np.asarray(inputs["dst"]).astype(np.int64)
    rel = np.asarray(inputs["rel_types"]).astype(np.int64)
    layout, idx_w, dstl_po = _preprocess_edges(src, dst, rel, cfg)
    key = (cfg.N, layout["NCOL"], debug,
           tuple(layout["nc_sym"].ravel().tolist()))
    if key not in _CACHE:
        _CACHE.clear()
        _CACHE[key] = build_program(cfg, layout, debug=debug)
    nc = _CACHE[key]
    in_maps = make_in_maps(inputs, cfg, layout, idx_w, dstl_po)
    res = bass_utils.run_bass_kernel_spmd(
        nc, in_maps, core_ids=list(range(cfg.cores)), trace=trace
    )
    last_results = res
    return res.results[0]["out"]


def kernel(**inputs):
    return _run(inputs, FULL_CFG, trace=False)


# revision 6
# speedup vs baseline: 1.2027x; 1.2027x over previous
"""RGCN (3 RelGraphConv layers + mean-pool + MLP + softmax) on 8 Trainium2 cores.

v2 strategy (dst-node sharding + fp8 DoubleRow matmuls):
  - Core c owns dst nodes [c*6250, (c+1)*6250), 25 tiles of 256.
  - h tables live in DRAM as fp8e4 rows padded to 256 B (dma_gather granularity).
  - Per tile, edges are grouped by (half, relation) into 128-slot columns with a
    SYMMETRIC layout: each relation gets the same number of columns in the A
    (src < thr) and B (src >= thr) blocks, so column j of A pairs with column j
    of B at a constant stride. One fp16 one-hot (DVE is_equal, fast 2x mode)
    serves both planes of a fp8 DoubleRow scatter matmul: the fp16 buffer is
    bitcast to fp8 byte pairs whose high bytes are 0x3C (= 1.5 in e4m3) at the
    one-hot positions. The 1.5 factor is divided out in the PSUM->SBUF copy.
  - The per-relation aggregates (fp8) hit the relation weights with DoubleRow
    matmuls too: relations are paired (0,1)(2,3)..(14,15)(loop,0) as the two
    contraction planes; W is host-packed fp8e4 (x32 prescale, undone in the
    ReLU activation scale) in plane-interleaved pairs.
  - dma_gather runs one call per (tile, half) chunk (single_packet=False,
    48 KB SWDGE ring) to amortize the 994 ns fixed SWDGE cost.
  - Layer-2 output stays fp16 and feeds weighted one-hot pooling matmuls
    accumulated in PSUM; AllReduce combines cores; transposed fp32 MLP +
    PE transpose + free-dim softmax emit [128, 8] on every core.
  - AllGather replicates fp8 node tables between conv layers.
"""

import sys

sys.path.insert(0, "/opt/trn_rl_repo")

import math
import numpy as np
import ml_dtypes

from concourse import bass, bacc, mybir, tile
from concourse import bass_utils

F32 = mybir.dt.float32
F16 = mybir.dt.float16
F8 = mybir.dt.float8e4
I16 = mybir.dt.int16
NPF8 = ml_dtypes.float8_e4m3

TS = 256          # dst nodes per tile
ROWB = 256        # fp8 elements per padded table row (256 B)
CHUNK_COLS = 22   # max columns per dma_gather call (ring = 3072 descs)
W_SCALE = 32.0    # host premultiplier on conv weights (undone in ReLU scale)
H_SCALE = 1.5     # e4m3 value of the one-hot's high byte (undone in at-copy)


class Cfg:
    def __init__(self, N, R, G, NC, cores, conv, mlp, split_cap=32768):
        self.N = N
        self.R = R
        self.G = G
        self.NC = NC
        self.cores = cores
        self.conv = conv
        self.mlp = mlp
        self.NPC = N // cores
        self.T = math.ceil(self.NPC / TS)
        self.split_cap = min(split_cap, N)
        self.baseB = max(0, N - self.split_cap)
        self.split_thr = min(max(N // 2, self.baseB), self.split_cap)


FULL_CFG = Cfg(
    N=50000, R=16, G=128, NC=8, cores=8,
    conv=[(128, 128), (128, 256), (256, 128)],
    mlp=[(128, 128), (128, 256), (256, 128)],
)


def _preprocess_edges(src, dst, rel, cfg):
    """Symmetric column layout + per-core slot arrays.

    Per tile: [A-cols r0..r16][B-cols r0..r16] with nc_sym[t,r] columns per
    relation IN EACH half (>=1), so A col j pairs with B col j at stride nA_t.
    """
    C, NPC, T, R = cfg.cores, cfg.NPC, cfg.T, cfg.R
    NG = R + 1
    rows_t = np.minimum(NPC - np.arange(T) * TS, TS).astype(np.int64)

    owner = dst // NPC
    isB = (src >= cfg.split_thr).astype(np.int64)
    tloc = (dst % NPC) // TS
    key_full = ((owner * T + tloc) * 2 + isB) * NG + rel
    cnt = np.bincount(key_full, minlength=C * T * 2 * NG).reshape(C, T, 2, NG)
    for t in range(T):
        for c in range(C):
            vs = c * NPC + t * TS + np.arange(int(rows_t[t]))
            nb = int((vs >= cfg.split_thr).sum())
            cnt[c, t, 0, R] = len(vs) - nb
            cnt[c, t, 1, R] = nb

    # symmetric per-half column count, min 1
    nc_sym = np.maximum(1, -(-cnt.max(axis=(0, 2)) // 128))  # [T, NG]
    a0 = np.zeros((T, NG), np.int64)  # local col offset of rel r's A block
    for t in range(T):
        a0[t] = np.concatenate([[0], np.cumsum(nc_sym[t])[:-1]])
    nA_t = nc_sym.sum(axis=1)  # [T]
    Ct_t = 2 * nA_t
    tile_base = np.zeros(T + 1, np.int64)
    tile_base[1:] = np.cumsum(Ct_t)
    NCOL = int(tile_base[-1])

    idx_po = np.zeros((C, 128, NCOL), np.int16)
    dstl_po = np.full((C, 128, NCOL), -1.0, np.float16)
    for c in range(C):
        m = owner == c
        es, ed, er = src[m], dst[m], rel[m]
        eb = (es >= cfg.split_thr).astype(np.int64)
        tl = (ed - c * NPC) // TS
        dl = (ed - c * NPC) % TS
        k = (tl * 2 + eb) * NG + er
        order = np.argsort(k, kind="stable")
        es, dl, k, eb = es[order], dl[order], k[order], eb[order]
        grp_start = np.searchsorted(k, np.arange(T * 2 * NG))
        j = np.arange(len(k)) - grp_start[k]
        kt, krem = k // (2 * NG), k % (2 * NG)
        kh, kr = krem // NG, krem % NG
        col = tile_base[kt] + kh * nA_t[kt] + a0[kt, kr] + j // 128
        p = j % 128
        val = np.where(eb == 1, es - cfg.baseB, es).astype(np.int16)
        idx_po[c, p, col] = val
        dstl_po[c, p, col] = dl.astype(np.float16)
        # self edges (r = R)
        for t in range(T):
            vl = np.arange(int(rows_t[t]))
            vg = c * NPC + t * TS + vl
            for h in range(2):
                sel = (vg >= cfg.split_thr) == (h == 1)
                if not sel.any():
                    continue
                vv, ll = vg[sel], vl[sel]
                cs = tile_base[t] + h * nA_t[t] + a0[t, R]
                jj = np.arange(len(vv))
                vval = vv - cfg.baseB if h == 1 else vv
                idx_po[c, jj % 128, cs + jj // 128] = vval.astype(np.int16)
                dstl_po[c, jj % 128, cs + jj // 128] = ll.astype(np.float16)

    # 16-wrapped idx array; per (t, half, chunk) call covers contiguous cols
    idx_w = np.zeros((C, 128, NCOL * 8), np.int16)
    calls = []  # (t, half, local_col0, ncols)
    for t in range(T):
        for h in range(2):
            blk0 = int(tile_base[t] + h * nA_t[t])
            nb = int(nA_t[t])
            for q0 in range(0, nb, CHUNK_COLS):
                calls.append((t, h, q0, min(CHUNK_COLS, nb - q0)))
            for c in range(C):
                flat = idx_po[c][:, blk0:blk0 + nb].T.ravel()
                blk = flat.reshape(-1, 16).T
                idx_w[c, :, blk0 * 8:blk0 * 8 + len(flat) // 16] = np.tile(
                    blk, (8, 1))

    layout = {
        "NCOL": NCOL,
        "nc_sym": nc_sym,        # [T, NG]
        "a0": a0,                # [T, NG]
        "nA_t": nA_t,
        "tile_base": tile_base,
        "rows_t": rows_t,
        "Cmax": int(Ct_t.max()),
        "calls": calls,
    }
    return layout, idx_w, dstl_po


def _pack_weights(inputs, cfg):
    """Conv weights: fp8e4, x32, relation-pair plane-interleaved.

    Wcv{l}: [ks, 128, 9*2*do] where pair q holds (W_{2q}, W_{2q+1}) for q<8
    and (W_loop, 0) for q=8, laid out [q][plane][do].
    """
    R = cfg.R
    packed = {}
    for l, (di, do) in enumerate(cfg.conv):
        ks = di // 128
        npair = (R + 2) // 2  # 9
        Wp = np.zeros((ks, 128, npair * 2 * do), NPF8)
        Wr = np.asarray(inputs[f"W_rel{l}"], np.float32) * W_SCALE
        Wl = np.asarray(inputs[f"W_loop{l}"], np.float32) * W_SCALE
        for k in range(ks):
            for q in range(npair):
                for tt in range(2):
                    r = 2 * q + tt
                    if r < R:
                        w = Wr[r, k * 128:(k + 1) * 128, :]
                    elif r == R:
                        w = Wl[k * 128:(k + 1) * 128, :]
                    else:
                        continue  # zero plane
                    Wp[k, :, (q * 2 + tt) * do:(q * 2 + tt + 1) * do] = (
                        w.astype(NPF8))
        packed[f"Wcv{l}"] = Wp
        packed[f"bcv{l}"] = (np.asarray(inputs[f"b{l}"], np.float32)
                             * W_SCALE).astype(np.float16).reshape(1, do)
    for l, (di, do) in enumerate(cfg.mlp):
        ks = di // 128
        W = np.asarray(inputs[f"Wh{l}"], np.float32)
        packed[f"Wm{l}"] = W.reshape(ks, 128, do)
        nmt = -(-do // 128)
        bp = np.zeros((nmt, 128, 1), np.float32)
        b = np.asarray(inputs[f"bh{l}"], np.float32)
        for mi in range(nmt):
            seg = b[mi * 128:(mi + 1) * 128]
            bp[mi, :len(seg), 0] = seg
        packed[f"bm{l}"] = bp
    packed["Wcls"] = np.asarray(inputs["Wc"], np.float32).reshape(1, 128, cfg.NC)
    packed["bcls"] = np.asarray(inputs["bc"], np.float32).reshape(cfg.NC, 1)
    return packed


def _pool_arrays(graph_ids, cfg):
    C, NPC = cfg.cores, cfg.NPC
    ST = math.ceil(NPC / 128)
    cnts = np.bincount(graph_ids, minlength=cfg.G).astype(np.float64)
    wg = (1.0 / np.maximum(cnts, 1.0)).astype(np.float32)
    gid_po = np.full((C, 128, ST), -1.0, np.float16)
    wnd_po = np.zeros((C, 128, ST), np.float32)
    for c in range(C):
        for st in range(ST):
            nt = min(128, NPC - st * 128)
            if nt <= 0:
                continue
            v = c * NPC + st * 128 + np.arange(nt)
            gid_po[c, :nt, st] = graph_ids[v].astype(np.float16)
            wnd_po[c, :nt, st] = wg[graph_ids[v]]
    return gid_po, wnd_po


def build_program(cfg, layout, debug=False, timing=False):
    nc = bacc.Bacc(
        "TRN2", target_bir_lowering=False, debug=False,
        enable_asserts=False, num_devices=cfg.cores,
        dynamic_dma_scratch_size=49152,
    )
    R, T, G, NC = cfg.R, cfg.T, cfg.G, cfg.NC
    NG = R + 1
    NPAIR = (R + 2) // 2
    NCOL, Cmax = layout["NCOL"], layout["Cmax"]
    nc_sym, a0 = layout["nc_sym"], layout["a0"]
    tile_base, nA_t = layout["tile_base"], layout["nA_t"]
    rows_t, calls = layout["rows_t"], layout["calls"]
    ST = math.ceil(cfg.NPC / 128)
    DR = mybir.MatmulPerfMode.DoubleRow

    h0 = nc.dram_tensor("h0", [cfg.N, ROWB], F8, kind="ExternalInput")
    idxT = nc.dram_tensor("idxw", [128, NCOL * 8], I16, kind="ExternalInput")
    dstlT = nc.dram_tensor("dstl", [128, NCOL * 16], F16, kind="ExternalInput")
    gidT = nc.dram_tensor("gid", [128, ST], F16, kind="ExternalInput")
    wndT = nc.dram_tensor("wnd", [128, ST], F32, kind="ExternalInput")
    iotaT = nc.dram_tensor("iota", [128, Cmax * TS], F16, kind="ExternalInput")
    onesT = nc.dram_tensor("ones1", [1, 128], F16, kind="ExternalInput")
    idcT = nc.dram_tensor("idc", [NC, NC], F32, kind="ExternalInput")
    WcvT, bcvT = [], []
    for l, (di, do) in enumerate(cfg.conv):
        WcvT.append(nc.dram_tensor(f"Wcv{l}", [di // 128, 128, NPAIR * 2 * do],
                                   F8, kind="ExternalInput"))
        bcvT.append(nc.dram_tensor(f"bcv{l}", [1, do], F16, kind="ExternalInput"))
    WmT, bmT = [], []
    for l, (di, do) in enumerate(cfg.mlp):
        WmT.append(nc.dram_tensor(f"Wm{l}", [di // 128, 128, do], F32,
                                  kind="ExternalInput"))
        bmT.append(nc.dram_tensor(f"bm{l}", [-(-do // 128), 128, 1], F32,
                                  kind="ExternalInput"))
    WclsT = nc.dram_tensor("Wcls", [1, 128, NC], F32, kind="ExternalInput")
    bclsT = nc.dram_tensor("bcls", [NC, 1], F32, kind="ExternalInput")
    outT = nc.dram_tensor("out", [G, NC], F32, kind="ExternalOutput")

    h_full = [h0]
    ag_in = []
    for l in range(2):
        ag_in.append(nc.dram_tensor(f"agin{l}", [cfg.NPC, ROWB], F8))
        h_full.append(nc.dram_tensor(f"hfull{l + 1}", [cfg.N, ROWB], F8))
    pool_in = nc.dram_tensor("plin", [128, G], F32)
    pool_out = nc.dram_tensor("plout", [128, G], F32)
    dbg = {}
    if debug:
        dbg["h1"] = nc.dram_tensor("dbg_h1", [cfg.N, ROWB], F8,
                                   kind="ExternalOutput")
        dbg["h2"] = nc.dram_tensor("dbg_h2", [cfg.N, ROWB], F8,
                                   kind="ExternalOutput")
        dbg["pool"] = nc.dram_tensor("dbg_pool", [128, G], F32,
                                     kind="ExternalOutput")

    rg = [list(range(cfg.cores))]
    calls_by_tile = {}
    for (t, h, q0, ncq) in calls:
        calls_by_tile.setdefault(t, []).append((h, q0, ncq))

    with tile.TileContext(nc) as tc:
        with (
            tc.tile_pool(name="const", bufs=1) as cp,
            tc.tile_pool(name="wp", bufs=1) as wp,
            tc.tile_pool(name="gp", bufs=3) as gp,
            tc.tile_pool(name="hp", bufs=2) as hp,
            tc.tile_pool(name="atp", bufs=3) as atp,
            tc.tile_pool(name="hnp", bufs=4) as hnp,
            tc.tile_pool(name="mp", bufs=2) as mp,
            tc.tile_pool(name="psA", bufs=2, space="PSUM") as psA,
            tc.tile_pool(name="psG", bufs=2, space="PSUM") as psG,
            tc.tile_pool(name="psP", bufs=1, space="PSUM") as psP,
        ):
            idx_sb = cp.tile([128, NCOL * 8], I16)
            nc.sync.dma_start(out=idx_sb[:], in_=idxT[:, :])
            dstl_sb = cp.tile([128, NCOL * 16], F16)
            nc.sync.dma_start(out=dstl_sb[:], in_=dstlT[:, :])
            iota_sb = cp.tile([128, Cmax * TS], F16)
            nc.sync.dma_start(out=iota_sb[:], in_=iotaT[:, :])
            gid_sb = cp.tile([128, ST], F16)
            nc.sync.dma_start(out=gid_sb[:], in_=gidT[:, :])
            wnd_sb = cp.tile([128, ST], F32)
            nc.sync.dma_start(out=wnd_sb[:], in_=wndT[:, :])
            ones_sb = cp.tile([1, 128], F16)
            nc.sync.dma_start(out=ones_sb[:], in_=onesT[:, :])
            idc_sb = cp.tile([NC, NC], F32)
            nc.sync.dma_start(out=idc_sb[:], in_=idcT[:, :])

            Wsb, bsb = [], []
            for l, (di, do) in enumerate(cfg.conv):
                ks = di // 128
                Wk = []
                for k in range(ks):
                    w = wp.tile([128, NPAIR * 2 * do], F8, tag=f"wcv{l}_{k}")
                    nc.sync.dma_start(out=w[:], in_=WcvT[l][k, :, :])
                    Wk.append(w)
                Wsb.append(Wk)
                b = wp.tile([1, do], F16, tag=f"bcv{l}")
                nc.sync.dma_start(out=b[:], in_=bcvT[l][:, :])
                bsb.append(b)
            Wm_sb, bm_sb = [], []
            for l, (di, do) in enumerate(cfg.mlp):
                ks = di // 128
                Wk = []
                for k in range(ks):
                    w = wp.tile([128, do], F32, tag=f"wm{l}_{k}")
                    nc.sync.dma_start(out=w[:], in_=WmT[l][k, :, :])
                    Wk.append(w)
                Wm_sb.append(Wk)
                nmt = -(-do // 128)
                bk = []
                for mi in range(nmt):
                    b = wp.tile([128, 1], F32, tag=f"bm{l}_{mi}")
                    nc.sync.dma_start(out=b[:], in_=bmT[l][mi, :, :])
                    bk.append(b)
                bm_sb.append(bk)
            Wcls_sb = wp.tile([128, NC], F32, tag="wcls")
            nc.sync.dma_start(out=Wcls_sb[:], in_=WclsT[0, :, :])
            bcls_sb = wp.tile([NC, 1], F32, tag="bcls")
            nc.sync.dma_start(out=bcls_sb[:], in_=bclsT[:, :])

            pool_ps = None

            for l, (di, do) in enumerate(cfg.conv):
                ks = di // 128
                gpb = 4 // ks  # groups per PSUM batch (pa = [128, 1024] f32)
                src_dram = h_full[l]
                tblA = src_dram[0:cfg.split_cap, :]
                tblB = src_dram[cfg.baseB:cfg.N, :]
                if l == 2:
                    pool_ps = psP.tile([128, G], F32, tag="pool")
                for t in range(T):
                    cb = int(tile_base[t])
                    nA = int(nA_t[t])
                    Ct = 2 * nA
                    rows = int(rows_t[t])
                    rows_ns = [min(128, rows), max(0, rows - 128)]

                    g_sb = gp.tile([128, Cmax * ROWB], F8, tag="g")
                    for (h, q0, ncq) in calls_by_tile[t]:
                        c_loc = h * nA + q0
                        n_idx = ncq * 128
                        o0 = (cb + c_loc) * 8
                        nc.gpsimd.dma_gather(
                            out_ap=g_sb[:, c_loc * ROWB:(c_loc + ncq) * ROWB]
                            .rearrange("p (c j) -> p c j", j=ROWB),
                            in_ap=(tblA if h == 0 else tblB),
                            idxs_ap=idx_sb[:, o0:o0 + n_idx // 16],
                            num_idxs=n_idx,
                            num_idxs_reg=n_idx,
                            elem_size=ROWB,
                            single_packet=False,
                        )
                    h_all = hp.tile([128, Cmax * TS], F16, tag="h")
                    nc.vector.tensor_tensor(
                        out=h_all[:, :Ct * TS].rearrange(
                            "p (c u v) -> p c u v", u=16, v=16),
                        in0=iota_sb[:, :Ct * TS].rearrange(
                            "p (c u v) -> p c u v", u=16, v=16),
                        in1=dstl_sb[:, cb * 16:(cb + Ct) * 16].rearrange(
                            "p (c u) -> p c u", u=16)[:, :, None, :]
                        .to_broadcast([128, Ct, 16, 16]),
                        op=mybir.AluOpType.is_equal,
                    )
                    # paired views: dim1 = half (A/B plane)
                    g2 = g_sb[:, :Ct * ROWB].rearrange(
                        "p (h x) -> p h x", h=2)
                    h8 = h_all[:, :Ct * TS].bitcast(F8).rearrange(
                        "p (h c v b) -> p h c v b", h=2, c=nA, v=TS, b=2)

                    agg = psG.tile([128, 512], F32, tag="agg")
                    for ns in range(2):
                        if rows_ns[ns] > 0:
                            nc.tensor.matmul(
                                out=agg[:, ns * do:ns * do + do],
                                lhsT=ones_sb[:1, :], rhs=bsb[l][:1, :],
                                start=True, stop=False,
                            )
                    bases = list(range(0, NG, gpb))
                    for bi, base in enumerate(bases):
                        bn = min(gpb, NG - base)
                        pa = psA.tile([128, 1024], F32, tag="pa")
                        for gi in range(bn):
                            r = base + gi
                            ncr = int(nc_sym[t, r])
                            ca0 = int(a0[t, r])
                            for k in range(ks):
                                for j in range(ncr):
                                    ca = ca0 + j
                                    nc.tensor.matmul(
                                        out=pa[:, gi * ks * TS + k * TS:
                                               gi * ks * TS + (k + 1) * TS],
                                        lhsT=g2[:, :, ca * ROWB + k * 128:
                                                ca * ROWB + k * 128 + 128],
                                        rhs=h8[:, :, ca, :, 1],
                                        start=(j == 0), stop=(j == ncr - 1),
                                        perf_mode=DR,
                                    )
                        wid = bn * ks * TS
                        at = atp.tile([128, 1024], F8, tag="at")
                        nc.scalar.activation(
                            out=at[:, :wid], in_=pa[:, :wid],
                            func=mybir.ActivationFunctionType.Copy,
                            scale=float(1.0 / H_SCALE),
                        )
                        at_v = at[:].rearrange("p (g x) -> p g x", x=ks * TS)
                        npair_b = (bn + 1) // 2
                        for q in range(npair_b):
                            qglob = (base + 2 * q) // 2
                            paired = 2 * q + 1 < bn
                            for ns in range(2):
                                if rows_ns[ns] == 0:
                                    continue
                                for k in range(ks):
                                    sl = k * TS + ns * 128
                                    if paired:
                                        lhsT = at_v[:, 2 * q:2 * q + 2,
                                                    sl:sl + 128]
                                    else:
                                        lhsT = at_v[:, 2 * q, sl:sl + 128][
                                            :, None, :].to_broadcast(
                                            [128, 2, 128])
                                    rhs = Wsb[l][k][:].rearrange(
                                        "p (q two d) -> p q two d",
                                        q=NPAIR, two=2)[:, qglob, :, :]
                                    last = (bi == len(bases) - 1
                                            and q == npair_b - 1
                                            and k == ks - 1)
                                    nc.tensor.matmul(
                                        out=agg[:, ns * do:ns * do + do],
                                        lhsT=lhsT, rhs=rhs,
                                        start=False, stop=last,
                                        perf_mode=DR,
                                    )
                    for ns in range(2):
                        rns = rows_ns[ns]
                        if rns == 0:
                            continue
                        st = t * 2 + ns
                        if l < 2:
                            hn = hnp.tile([128, 256], F8, tag="hn")
                            nc.scalar.activation(
                                out=hn[:, :do], in_=agg[:, ns * do:ns * do + do],
                                func=mybir.ActivationFunctionType.Relu,
                                scale=float(1.0 / W_SCALE),
                            )
                            nc.sync.dma_start(
                                out=ag_in[l][st * 128:st * 128 + rns, 0:do],
                                in_=hn[:rns, :do],
                            )
                        else:
                            hn16 = hnp.tile([128, 128], F16, tag="hn16")
                            nc.scalar.activation(
                                out=hn16[:, :do],
                                in_=agg[:, ns * do:ns * do + do],
                                func=mybir.ActivationFunctionType.Relu,
                                scale=float(1.0 / W_SCALE),
                            )
                            hg = mp.tile([128, G], F16, tag="hg")
                            nc.vector.tensor_tensor(
                                out=hg[:],
                                in0=iota_sb[:, :G],
                                in1=gid_sb[:, st:st + 1].to_broadcast([128, G]),
                                op=mybir.AluOpType.is_equal,
                            )
                            nc.vector.tensor_scalar_mul(
                                out=hg[:], in0=hg[:], scalar1=wnd_sb[:, st:st + 1]
                            )
                            nc.tensor.matmul(
                                out=pool_ps[:], lhsT=hn16[:, :do], rhs=hg[:],
                                start=(st == 0), stop=(st == 2 * T - 2),
                            )
                if l < 2:
                    if timing:
                        nc.sync.dma_start(
                            out=h_full[l + 1][0:cfg.NPC, :], in_=ag_in[l][:, :]
                        )
                    else:
                        nc.gpsimd.collective_compute(
                            "AllGather",
                            mybir.AluOpType.bypass,
                            replica_groups=rg,
                            ins=[ag_in[l].ap().opt()],
                            outs=[h_full[l + 1].ap().opt()],
                        )
                    if debug:
                        nc.sync.dma_start(
                            out=dbg[f"h{l + 1}"][:, :], in_=h_full[l + 1][:, :]
                        )

            # ---- pooled AllReduce + MLP (transposed, fp32) ----
            pl_sb = mp.tile([128, G], F32, tag="pl")
            nc.vector.tensor_copy(out=pl_sb[:], in_=pool_ps[:])
            nc.sync.dma_start(out=pool_in[:, :], in_=pl_sb[:])
            if timing:
                nc.sync.dma_start(out=pool_out[:, :], in_=pool_in[:, :])
            else:
                nc.gpsimd.collective_compute(
                    "AllReduce",
                    mybir.AluOpType.add,
                    replica_groups=rg,
                    ins=[pool_in.ap().opt()],
                    outs=[pool_out.ap().opt()],
                )
            hgT = mp.tile([128, G], F32, tag="hgt")
            nc.sync.dma_start(out=hgT[:], in_=pool_out[:, :])
            if debug:
                nc.sync.dma_start(out=dbg["pool"][:, :], in_=pool_out[:, :])

            cur = [hgT]
            for l, (di, do) in enumerate(cfg.mlp):
                ks = di // 128
                nmt = -(-do // 128)
                nxt = []
                for mi in range(nmt):
                    mw = min(128, do - mi * 128)
                    ps = psG.tile([128, 512], F32, tag="agg")
                    for k in range(ks):
                        nc.tensor.matmul(
                            out=ps[:mw, :G],
                            lhsT=Wm_sb[l][k][:, mi * 128:mi * 128 + mw],
                            rhs=cur[k][:],
                            start=(k == 0), stop=(k == ks - 1),
                        )
                    hx = mp.tile([128, G], F32, tag=f"mlph{l}_{mi}")
                    nc.scalar.activation(
                        out=hx[:mw, :], in_=ps[:mw, :G],
                        func=mybir.ActivationFunctionType.Relu,
                        bias=bm_sb[l][mi][:mw, :1],
                    )
                    nxt.append(hx)
                cur = nxt

            ps_cls = psG.tile([128, 512], F32, tag="agg")
            nc.tensor.matmul(
                out=ps_cls[:NC, :G], lhsT=Wcls_sb[:, :NC], rhs=cur[0][:],
                start=True, stop=True,
            )
            lgT = mp.tile([NC, G], F32, tag="lgT")
            nc.vector.tensor_scalar_add(
                out=lgT[:], in0=ps_cls[:NC, :G], scalar1=bcls_sb[:, :1]
            )
            ps_tr = psG.tile([128, 512], F32, tag="agg")
            nc.tensor.transpose(out=ps_tr[:G, :NC], in_=lgT[:], identity=idc_sb[:])
            lg = mp.tile([G, NC], F32, tag="lg")
            nc.vector.tensor_copy(out=lg[:], in_=ps_tr[:G, :NC])
            mx = mp.tile([G, 1], F32, tag="mx")
            nc.vector.tensor_reduce(
                out=mx[:], in_=lg[:], axis=mybir.AxisListType.X,
                op=mybir.AluOpType.max,
            )
            nc.vector.tensor_scalar_mul(out=mx[:], in0=mx[:], scalar1=-1.0)
            ex = mp.tile([G, NC], F32, tag="ex")
            nc.scalar.activation(
                out=ex[:], in_=lg[:], func=mybir.ActivationFunctionType.Exp,
                bias=mx[:, :1],
            )
            sm = mp.tile([G, 1], F32, tag="sm")
            nc.vector.tensor_reduce(
                out=sm[:], in_=ex[:], axis=mybir.AxisListType.X,
                op=mybir.AluOpType.add,
            )
            rs = mp.tile([G, 1], F32, tag="rs")
            nc.vector.reciprocal(out=rs[:], in_=sm[:])
            ot = mp.tile([G, NC], F32, tag="ot")
            nc.vector.tensor_scalar_mul(out=ot[:], in0=ex[:], scalar1=rs[:, :1])
            nc.sync.dma_start(out=outT[:, :], in_=ot[:])

    nc.compile()
    return nc


def make_in_maps(inputs, cfg, layout, idx_w, dstl_po):
    gid_po, wnd_po = _pool_arrays(
        np.asarray(inputs["graph_ids"]).astype(np.int64), cfg
    )
    packed = _pack_weights(inputs, cfg)
    Cmax = max(layout["Cmax"], 1)
    iota = np.tile(np.arange(TS, dtype=np.float16)[None, :], (128, Cmax))
    iota = iota.reshape(128, Cmax * TS)
    h8 = np.zeros((cfg.N, ROWB), NPF8)
    h8[:, :cfg.conv[0][0]] = np.asarray(inputs["h"], np.float32).astype(NPF8)
    shared = {
        "h0": h8,
        "iota": iota,
        "ones1": np.ones((1, 128), np.float16),
        "idc": np.eye(cfg.NC, dtype=np.float32),
    }
    shared.update(packed)
    in_maps = []
    for c in range(cfg.cores):
        m = dict(shared)
        m["idxw"] = idx_w[c]
        m["dstl"] = np.repeat(dstl_po[c], 16, axis=1)
        m["gid"] = gid_po[c]
        m["wnd"] = wnd_po[c]
        in_maps.append(m)
    return in_maps


_CACHE = {}
last_results = None


def _run(inputs, cfg, trace=False, debug=False):
    global last_results
    src = np.asarray(inputs["src"]).astype(np.int64)
    dst = np.asarray(inputs["dst"]).astype(np.int64)
    rel = np.asarray(inputs["rel_types"]).astype(np.int64)
    layout, idx_w, dstl_po = _preprocess_edges(src, dst, rel, cfg)
    key = (cfg.N, layout["NCOL"], debug,
           tuple(layout["nc_sym"].ravel().tolist()))
    if key not in _CACHE:
        _CACHE.clear()
        _CACHE[key] = build_program(cfg, layout, debug=debug)
    nc = _CACHE[key]
    in_maps = make_in_maps(inputs, cfg, layout, idx_w, dstl_po)
    res = bass_utils.run_bass_kernel_spmd(
        nc, in_maps, core_ids=list(range(cfg.cores)), trace=trace
    )
    last_results = res
    return res.results[0]["out"]


def kernel(**inputs):
    return _run(inputs, FULL_CFG, trace=False)
